# revision 28
# baseline (speedup 1.0000x reference)
"""Trainium2 Bass kernel: GPT-2 style causal attention + output projection.

Reference computation (B=2, L=2048, D=1024, H=16, dh=64):
    q,k,v = split_heads(query/key/value)            # [B,H,L,dh]
    S = q @ k^T / sqrt(dh)                          # [B,H,L,L]
    P = softmax(causal_mask(S))
    A = merge_heads(P @ v)                          # [B,L,D]
    out = A @ w_proj + b_proj

Sharding: 32 (b,h) pairs, 4 per core (cores 0-3 batch 0, 4-7 batch 1).
Each core computes attention for its 4 heads fully causally and a partial
c_proj using its 256 rows of w_proj; the host sums the 4 partials per batch.

Device-side layout trick: scores are computed transposed (S^T, keys on
partitions) so softmax's P lands with keys on the partition axis, which is
exactly the contraction layout P.V needs -- no on-device transposes anywhere.
A ones-column appended to V makes the same matmul emit softmax denominators.

Scheduling: the per-chunk dependency chain QK -> exp(ACT) -> mask(DVE) -> PV
is ~1.7us long while the PE work per chunk is <1us, and the PE executes its
queue in order.  QK therefore runs two chunks ahead of PV (3 score buffers),
and each q-block's c_proj is deferred into the next block's chunk stream so
the PE never sits behind a semaphore that recent work hasn't satisfied yet.

Precision: all matmul operands in fp16 (FWL-eligible weights, negligible
rounding at 10 mantissa bits); fp32 PSUM accumulation; softmax in fp32.
"""

import numpy as np

B, L, D, H = 2, 2048, 1024, 16
DH = 64          # head dim
PAIRS = 4        # (b,h) pairs per core
QB = 512         # query block
KC = 128         # key chunk
NCORES = 8

_COMPILED = None


def _build_nc():
    import concourse.bacc as bacc
    import concourse.tile as tile
    from concourse import mybir

    f32 = mybir.dt.float32
    f32r = mybir.dt.float32r
    bf16 = mybir.dt.bfloat16
    f16 = mybir.dt.float16
    Exp = mybir.ActivationFunctionType.Exp

    nc = bacc.Bacc("TRN2", target_bir_lowering=False, debug=False,
                   num_devices=NCORES)

    qt_d = nc.dram_tensor("qt", [2, 128, L], f16, kind="ExternalInput").ap()
    kt_d = nc.dram_tensor("kt", [2, 128, L], f16, kind="ExternalInput").ap()
    v_d = nc.dram_tensor("v", [PAIRS, 128, (L // KC) * 128], f16,
                         kind="ExternalInput").ap()
    masks_d = nc.dram_tensor("masks", [128, 4 * 1024], f16, kind="ExternalInput").ap()
    w_d = nc.dram_tensor("w", [2, 128, D], f16, kind="ExternalInput").ap()
    ones_d = nc.dram_tensor("ones", [1, DH], f32r, kind="ExternalInput").ap()
    out_d = nc.dram_tensor("out", [L, D], f32, kind="ExternalOutput").ap()

    with tile.TileContext(nc) as tc:
        with (
            tc.tile_pool(name="consts", bufs=1) as consts,
            tc.tile_pool(name="st", bufs=3, space="PSUM") as st_pool,
            tc.tile_pool(name="at", bufs=2, space="PSUM") as at_pool,
            tc.tile_pool(name="et", bufs=6) as et_pool,
            tc.tile_pool(name="atn", bufs=6) as atn_pool,
            tc.tile_pool(name="dsb", bufs=4) as dsb_pool,
            tc.tile_pool(name="rbc", bufs=4) as rbc_pool,
            tc.tile_pool(name="osb", bufs=4) as osb_pool,
        ):
            # resident inputs
            qt = [consts.tile([128, L], f16, name=f"qt{i}", tag=f"qt{i}")
                  for i in range(2)]
            kt = [consts.tile([128, L], f16, name=f"kt{i}", tag=f"kt{i}")
                  for i in range(2)]
            vt = [consts.tile([128, (L // KC) * 128], f16,
                              name=f"vt{i}", tag=f"vt{i}") for i in range(PAIRS)]
            mk = consts.tile([128, 4 * 1024], f16, name="mk", tag="mk")
            wt = [consts.tile([128, D], f16, name=f"wt{i}", tag=f"wt{i}")
                  for i in range(2)]
            ones = consts.tile([1, DH], f32r, name="ones", tag="ones")

            # startup loads spread over four DMA rings; the J=3 q-block of
            # duo 0 runs first, so its kt/qt slices land first
            nc.sync.dma_start(kt[0][:, 0:256], kt_d[0][:, 0:256])
            nc.scalar.dma_start(qt[0][:, 1536:2048], qt_d[0][:, 1536:2048])
            for p in range(2):
                nc.sync.dma_start(vt[p][:], v_d[p])
            nc.sync.dma_start(kt[0][:, 256:2048], kt_d[0][:, 256:2048])
            nc.scalar.dma_start(qt[0][:, 0:1536], qt_d[0][:, 0:1536])
            nc.scalar.dma_start(qt[1][:], qt_d[1])
            nc.sync.dma_start(mk[:], masks_d[:])
            nc.sync.dma_start(kt[1][:], kt_d[1])
            for p in range(2, PAIRS):
                nc.sync.dma_start(vt[p][:], v_d[p])
            nc.scalar.dma_start(ones[:], ones_d[:])
            for i in range(2):
                nc.scalar.dma_start(wt[i][:], w_d[i])

            def cproj_groups(J, atn_duo, final=False):
                def one(rt, nf):
                    def emit():
                        cp = st_pool.tile([128, 512], f32, name="cp", tag="st")
                        for duo in range(2):
                            nc.tensor.matmul(
                                cp[:],
                                lhsT=atn_duo[duo][:, rt * 128:(rt + 1) * 128],
                                rhs=wt[duo][:, nf * 512:(nf + 1) * 512],
                                start=(duo == 0), stop=(duo == 1),
                            )
                        ob = osb_pool.tile([128, 512], f32, name="ob", tag="ob")
                        if final and rt % 2:
                            nc.scalar.copy(ob[:], cp[:])
                        else:
                            nc.vector.tensor_copy(ob[:], cp[:])
                        nc.sync.dma_start(
                            out_d[J * QB + rt * 128:J * QB + (rt + 1) * 128,
                                  nf * 512:(nf + 1) * 512],
                            ob[:],
                        )
                    return emit
                return [one(rt, nf)
                        for rt in range(QB // 128) for nf in range(2)]

            pending = []             # c_proj groups from the previous q-block
            pending_norm = []        # softmax-normalize chains, deferred into
                                     # the next duo's chunk stream
            pending_pv = []          # tail PVs of the previous duo
            for J in reversed(range(L // QB)):
                nch = 4 * J + 4      # causal: key chunks 0..nch-1
                atn_duo = []
                for duo in range(2):
                    at = [at_pool.tile([128, QB], f32, name="at", tag="at")
                          for _ in range(2)]
                    ets = {}

                    def emit_qk(c):
                        st = st_pool.tile([128, 2 * QB], f32, name="st",
                                          tag="st")
                        for h2 in range(2):
                            nc.tensor.matmul(
                                st[:, h2 * QB:(h2 + 1) * QB],
                                lhsT=kt[duo][64 * h2:64 * (h2 + 1),
                                             c * KC:(c + 1) * KC],
                                rhs=qt[duo][64 * h2:64 * (h2 + 1),
                                            J * QB:(J + 1) * QB],
                                start=True, stop=True,
                                tile_position=(64 * h2, 0),
                            )
                        et = et_pool.tile([128, 2 * QB], f16, name="et",
                                          tag="et")
                        m = c - 4 * J
                        if m >= 1:
                            # diagonal chunk: only q >= m*128 within each pair
                            # half is causally valid; the mask zeroes the
                            # stale rest (every et slot was fully written
                            # during J=3, which runs first)
                            nc.scalar.activation(
                                et[:].rearrange("p (h q) -> p h q", h=2)
                                [:, :, m * KC:],
                                st[:].rearrange("p (h q) -> p h q", h=2)
                                [:, :, m * KC:],
                                Exp, scale=0.125)
                        else:
                            nc.scalar.activation(et[:], st[:], Exp, scale=0.125)
                        if m >= 0:
                            # only q < (m+1)*128 needs masking/zeroing; beyond
                            # the diagonal band every key in this chunk is
                            # causally valid
                            w = (m + 1) * KC
                            ev = et[:].rearrange("p (h q) -> p h q", h=2)[:, :, 0:w]
                            mv = mk[:].rearrange("p (m h q) -> p m h q",
                                                 m=4, h=2)[:, m, :, 0:w]
                            nc.vector.tensor_mul(ev, ev, mv)
                        ets[c] = et

                    def emit_pv(c, ets=ets, at=at, duo=duo, nch=nch):
                        et = ets.pop(c)
                        for h2 in range(2):
                            pair = 2 * duo + h2
                            nc.tensor.matmul(
                                at[h2][0:128, :],
                                lhsT=vt[pair][:, c * 128:(c + 1) * 128],
                                rhs=et[:, h2 * QB:(h2 + 1) * QB],
                                start=(c == 0), stop=(c == nch - 1),
                            )

                    for c in range(nch):
                        emit_qk(c)
                        if pending_pv:
                            pending_pv.pop(0)()
                        elif pending_norm:
                            pending_norm.pop(0)()
                        elif (c >= 2 and pending
                              and (nch < 12 or c % 2 == 0)):
                            pending.pop(0)()
                        if J == 0 and duo == 1 and pending:
                            # last q-block, second duo: every prior block's
                            # normalize chain is already emitted, so drain
                            # leftovers before they pile onto the kernel tail
                            pending.pop(0)()
                        if c >= 2:
                            emit_pv(c - 2)
                    # defer this duo's tail PVs into the next duo's stream so
                    # the next QKs (and their exps) issue without waiting on
                    # the exp->mask->PV chain of this duo's last chunks
                    pending_pv.extend(
                        [lambda c=nch - 2, f=emit_pv: f(c),
                         lambda c=nch - 1, f=emit_pv: f(c)])

                    atn = atn_pool.tile([128, QB], f16, name="atn", tag="atn")

                    def norm_one(h2, at=at, atn=atn):
                        def emit():
                            dsb = dsb_pool.tile([1, QB], f32r, name="dsb",
                                                tag="dsb")
                            nc.vector.tensor_copy(dsb[:], at[h2][64:65, :])
                            bc = st_pool.tile([64, QB], f32, name="bc",
                                              tag="st")
                            nc.tensor.matmul(bc[:], lhsT=ones[:], rhs=dsb[:],
                                             start=True, stop=True)
                            rbc = rbc_pool.tile([64, QB], f32, name="rbc",
                                                tag="rbc")
                            nc.vector.reciprocal_approx_fast(rbc[:], bc[:])
                            nc.vector.tensor_mul(
                                atn[64 * h2:64 * (h2 + 1), :],
                                at[h2][0:64, :], rbc[:])
                        return emit

                    pending_norm.extend([norm_one(0), norm_one(1)])
                    atn_duo.append(atn)

                for g in pending:       # any stragglers from a short q-block
                    g()
                pending = cproj_groups(J, atn_duo, final=(J == 0))
            for g in pending_pv:
                g()
            for g in pending_norm:
                g()
            for g in pending:
                g()

    nc.compile()
    return nc


def _get_nc():
    global _COMPILED
    if _COMPILED is None:
        _COMPILED = _build_nc()
    return _COMPILED


def _prep_in_maps(query, key, value, w_proj):
    import ml_dtypes

    q = np.asarray(query, dtype=np.float32)
    k = np.asarray(key, dtype=np.float32)
    v = np.asarray(value, dtype=np.float32)
    w = np.asarray(w_proj, dtype=np.float32)

    q4 = q.reshape(B, L, H, DH)
    k4 = k.reshape(B, L, H, DH)
    v4 = v.reshape(B, L, H, DH)

    kp = np.arange(128)[:, None]
    qf = np.arange(QB)[None, :]
    mk_parts = []
    for m in range(4):
        mm = (kp + 128 * m <= qf).astype(np.float32)        # [128, 512]
        mk_parts.append(np.concatenate([mm, mm], axis=1))    # [128, 1024]
    masks = np.ascontiguousarray(
        np.concatenate(mk_parts, axis=1).astype(np.float16))
    ones64 = np.ones((1, DH), dtype=np.float32)

    in_maps = []
    for c in range(NCORES):
        b = c // 4
        hsel = 4 * (c % 4)
        qt = np.ascontiguousarray(
            q4[b].transpose(1, 2, 0)[hsel:hsel + 4].reshape(2, 128, L)
            .astype(np.float16))
        kt = np.ascontiguousarray(
            k4[b].transpose(1, 2, 0)[hsel:hsel + 4].reshape(2, 128, L)
            .astype(np.float16))
        vsl = v4[b, :, hsel:hsel + 4, :].transpose(1, 0, 2)  # [4, L, DH]
        vext = np.concatenate(
            [vsl, np.ones((PAIRS, L, 1), dtype=np.float32),
             np.zeros((PAIRS, L, 128 - DH - 1), dtype=np.float32)], axis=2)
        # pre-swizzle to the SBUF layout: [pair, partition, chunk*128]
        vext = (vext.reshape(PAIRS, L // KC, KC, 128)
                .transpose(0, 2, 1, 3).reshape(PAIRS, KC, -1))
        vext = np.ascontiguousarray(vext.astype(np.float16))
        wp = np.ascontiguousarray(
            w[(c % 4) * 256:(c % 4 + 1) * 256, :].reshape(2, 128, D)
            .astype(np.float16))
        in_maps.append({"qt": qt, "kt": kt, "v": vext, "masks": masks,
                        "w": wp, "ones": ones64})
    return in_maps


def kernel(query, key, value, w_proj, b_proj, n_head):
    from concourse.bass_utils import run_bass_kernel_spmd

    bias = np.asarray(b_proj, dtype=np.float32)
    in_maps = _prep_in_maps(query, key, value, w_proj)
    nc = _get_nc()
    res = run_bass_kernel_spmd(nc, in_maps, list(range(NCORES)))

    out = np.zeros((B, L, D), dtype=np.float32)
    for c in range(NCORES):
        out[c // 4] += res.results[c]["out"]
    out += bias[None, None, :]
    return out
